# revision 5
# baseline (speedup 1.0000x reference)
"""Radial power-spectrum (GroupStat.get_spectrum) Trainium2 kernel.

Math:  out[b,c,r] = sum_{p: idx[p]==r} x[b,c,p]^2 * w[p] / (cnt[r]+eps)

Strategy (8 NeuronCores, sharded over PIXELS, not batch):
  * All B*C = 1024 (b,c) rows on every core; each core owns ~1/8 of the
    33024 pixels (padded to 8*4224 = 33792, pad weight 0).
  * Host prep: transpose x to pixel-major [NPIX, 1024], scale by 32 and
    cast to fp16.  With 1024 rows per pixel the DMA lines are 2 KB, so
    the load runs at full HBM bandwidth AND lands with pixel on the
    partition dim -- no on-device transpose at all.
  * Device pipeline per 128-pixel chunk (33 per core):
      - DMA fp16 tiles [128p, 4, 1024n] (4 chunks per DMA; last tile is
        a single chunk so the post-DMA tail is short)
      - square in fp16 (values are 1024*x^2; the 32x host prescale keeps
        tiny x^2 out of fp16 subnormals), split between ScalarE and DVE
      - DVE: weighted one-hot [128p, 130r] = (iota == idx[p]) * wt[p],
        built ONCE per chunk and reused by all 8 row-groups
      - PE: for each of 8 row-groups g: psum_g[128n,130r] += x2T_g @ oh
  * psum_g -> SBUF f32 (4 copies on ScalarE, 4 on DVE, concurrently),
    two output DMAs overlap the copies; host sums the 8 per-core
    partials (pixel sharding => partial shell sums) and divides by 1024.
"""

import numpy as np

from concourse import bass, bacc, mybir
import concourse.tile as tile
from concourse.bass_utils import run_bass_kernel_spmd

B, C, S, XDIM = 128, 8, 256, 129
MAX_R = XDIM                # 129 shells
EPS = 1e-5
NCORES = 8
NROW = B * C                # 1024 total (b,c) rows
NGRP = NROW // 128          # 8 row-groups of 128
NPIX = S * XDIM             # 33024 pixels
NCH = 33                    # chunks of 128 pixels per core
CPIX = NCH * 128            # 4224 pixels per core
NPIX_PAD = NCORES * CPIX    # 33792
RPAD = 130                  # even free dim for DVE 4x mode; col 129 unused
TILES = [4] * 8 + [1]       # chunks per DMA tile (sum = 33)
PRESCALE = 32.0             # host multiplies x by 32 -> squares are 1024*x^2

F32 = mybir.dt.float32
F16 = mybir.dt.float16

_CACHE: dict = {}


def _build_program():
    nc = bacc.Bacc("TRN2", target_bir_lowering=False, debug=False,
                   num_devices=NCORES)

    # x, pre-transposed+scaled+fp16 on host: [chunk, pixel-in-chunk, row]
    x_d = nc.dram_tensor("xt", [NCH, 128, NROW], F16,
                         kind="ExternalInput").ap()
    # idx and wt packed: col c = idx for chunk c, col NCH+c = wt for chunk c
    iw_d = nc.dram_tensor("iw", [128, 2 * NCH], F32,
                          kind="ExternalInput").ap()
    iota_d = nc.dram_tensor("iota", [128, RPAD], F16,
                            kind="ExternalInput").ap()
    out_d = nc.dram_tensor("out", [128, NGRP * MAX_R], F32,
                           kind="ExternalOutput").ap()

    with tile.TileContext(nc) as tc:
        with tc.tile_pool(name="const", bufs=1) as const_pool, \
             tc.tile_pool(name="xin", bufs=3) as xin_pool, \
             tc.tile_pool(name="x2", bufs=3) as x2_pool, \
             tc.tile_pool(name="oh", bufs=8) as oh_pool, \
             tc.tile_pool(name="acc", bufs=1, space="PSUM") as acc_pool:

            accs = [acc_pool.tile([128, RPAD], F32, name=f"acc{g}")
                    for g in range(NGRP)]
            iw_t = const_pool.tile([128, 2 * NCH], F32)
            iota_t = const_pool.tile([128, RPAD], F16)

            c0 = 0
            first = True
            for tch in TILES:
                xin = xin_pool.tile([128, 4, NROW], F16, tag="xin")
                nc.sync.dma_start(
                    xin[:, :tch], x_d[c0:c0 + tch].rearrange("c p n -> p c n"))
                if first:
                    # consts slot in behind the first big load
                    nc.sync.dma_start(iw_t[:], iw_d[:])
                    nc.sync.dma_start(iota_t[:], iota_d[:])
                    first = False
                x2 = x2_pool.tile([128, 4, NROW], F16, tag="x2")
                flat_in = xin[:, :tch].rearrange("p c n -> p (c n)")
                flat_out = x2[:, :tch].rearrange("p c n -> p (c n)")
                half = (tch * NROW) // 2
                nc.scalar.activation(flat_out[:, :half], flat_in[:, :half],
                                     mybir.ActivationFunctionType.Square)
                nc.vector.tensor_tensor(flat_out[:, half:], flat_in[:, half:],
                                        flat_in[:, half:],
                                        op=mybir.AluOpType.mult)
                for j in range(tch):
                    c = c0 + j
                    oh = oh_pool.tile([128, RPAD], F16, tag="oh")
                    nc.vector.tensor_scalar(
                        oh[:], iota_t[:],
                        scalar1=iw_t[:, c:c + 1],
                        scalar2=iw_t[:, NCH + c:NCH + c + 1],
                        op0=mybir.AluOpType.is_equal,
                        op1=mybir.AluOpType.mult)
                    for g in range(NGRP):
                        nc.tensor.matmul(accs[g][:],
                                         lhsT=x2[:, j, g * 128:(g + 1) * 128],
                                         rhs=oh[:],
                                         start=(c == 0), stop=(c == NCH - 1))
                c0 += tch

            # psum -> sbuf fp16: groups 0-3 on ScalarE, 4-7 on DVE (parallel);
            # each half gets its own output DMA so the second overlaps copies
            res = const_pool.tile([128, NGRP * MAX_R], F32)
            for g in range(4):
                nc.scalar.copy(res[:, g * MAX_R:(g + 1) * MAX_R],
                               accs[g][:, :MAX_R])
            for g in range(4, NGRP):
                nc.vector.tensor_copy(res[:, g * MAX_R:(g + 1) * MAX_R],
                                      accs[g][:, :MAX_R])
            h = 4 * MAX_R
            nc.sync.dma_start(out_d[:, :h], res[:, :h])
            nc.sync.dma_start(out_d[:, h:], res[:, h:])

    nc.compile()
    return nc


def _get_program():
    if "nc" not in _CACHE:
        _CACHE["nc"] = _build_program()
    return _CACHE["nc"]


def _host_prep(shell_index: np.ndarray, shells_weight: np.ndarray,
               shells_count: np.ndarray):
    idx_flat = shell_index.reshape(-1).astype(np.int64)
    wt = shells_weight.reshape(-1).astype(np.float64) / (
        shells_count.astype(np.float64)[idx_flat] + EPS)
    idx_pad = np.zeros(NPIX_PAD, np.float32)
    idx_pad[:NPIX] = idx_flat
    wt_pad = np.zeros(NPIX_PAD, np.float32)
    wt_pad[:NPIX] = wt
    # per-core packed [idx | wt], chunk-transposed: A[i, c] = v[c*128 + i]
    iw = []
    for k in range(NCORES):
        sl = slice(k * CPIX, (k + 1) * CPIX)
        iw.append(np.concatenate(
            [idx_pad[sl].reshape(NCH, 128).T,
             wt_pad[sl].reshape(NCH, 128).T], axis=1).astype(np.float32))
    iota = np.broadcast_to(np.arange(RPAD, dtype=np.float16),
                           (128, RPAD)).copy()
    return iw, iota


def kernel(x: np.ndarray, shell_index: np.ndarray,
           shells_weight: np.ndarray, shells_count: np.ndarray,
           _trace: bool = False, **_tr_kwargs) -> np.ndarray:
    assert x.shape == (B, C, S, XDIM)
    nc = _get_program()
    iw, iota = _host_prep(shell_index, shells_weight, shells_count)

    x16 = (x.reshape(NROW, NPIX) * np.float32(PRESCALE)).astype(np.float16)
    in_maps = []
    for k in range(NCORES):
        lo = k * CPIX
        hi = min((k + 1) * CPIX, NPIX)
        xk = np.zeros((CPIX, NROW), np.float16)
        xk[:hi - lo] = x16[:, lo:hi].T
        in_maps.append({"xt": xk.reshape(NCH, 128, NROW), "iw": iw[k],
                        "iota": iota})

    res = run_bass_kernel_spmd(nc, in_maps, list(range(NCORES)),
                               trace=_trace, **_tr_kwargs)
    # each core returns [128, 8*129] f32 partial (1024x scaled) shell sums
    parts = np.stack([res.results[k]["out"] for k in range(NCORES)])
    full = parts.astype(np.float64).sum(axis=0) / (PRESCALE * PRESCALE)
    full = full.reshape(128, NGRP, MAX_R)
    # row-group g holds global rows g*128..(g+1)*128-1
    full = np.ascontiguousarray(full.transpose(1, 0, 2)).reshape(
        NROW, MAX_R).astype(np.float32)
    full = full.reshape(B, C, MAX_R)
    if _trace:
        return full, res
    return full


# revision 8
# speedup vs baseline: 1.0149x; 1.0149x over previous
"""Radial power-spectrum (GroupStat.get_spectrum) Trainium2 kernel.

Math:  out[b,c,r] = sum_{p: idx[p]==r} x[b,c,p]^2 * w[p] / (cnt[r]+eps)

Strategy (8 NeuronCores, sharded over PIXELS, not batch):
  * All B*C = 1024 (b,c) rows on every core; each core owns ~1/8 of the
    33024 pixels (padded to 8*4224 = 33792, pad weight 0).
  * Host prep: transpose x to pixel-major [NPIX, 1024], scale by 32 and
    cast to fp16.  With 1024 rows per pixel the DMA lines are 2 KB, so
    the load runs at full HBM bandwidth AND lands with pixel on the
    partition dim -- no on-device transpose at all.
  * Device pipeline per 128-pixel chunk (33 per core):
      - DMA fp16 tiles [128p, 4, 1024n] (4 chunks per DMA; last tile is
        a single chunk so the post-DMA tail is short)
      - square in fp16 (values are 1024*x^2; the 32x host prescale keeps
        tiny x^2 out of fp16 subnormals), split between ScalarE and DVE
      - DVE: weighted one-hot [128p, 130r] = (iota == idx[p]) * wt[p],
        built ONCE per chunk and reused by all 8 row-groups
      - PE: for each of 8 row-groups g: psum_g[128n,130r] += x2T_g @ oh
  * psum_g -> SBUF f32 (4 copies on ScalarE, 4 on DVE, concurrently),
    two output DMAs overlap the copies; host sums the 8 per-core
    partials (pixel sharding => partial shell sums) and divides by 1024.
"""

import numpy as np

from concourse import bass, bacc, mybir
import concourse.tile as tile
from concourse.bass_utils import run_bass_kernel_spmd

B, C, S, XDIM = 128, 8, 256, 129
MAX_R = XDIM                # 129 shells
EPS = 1e-5
NCORES = 8
NROW = B * C                # 1024 total (b,c) rows
NGRP = NROW // 128          # 8 row-groups of 128
NPIX = S * XDIM             # 33024 pixels
NCH = 33                    # chunks of 128 pixels per core
CPIX = NCH * 128            # 4224 pixels per core
NPIX_PAD = NCORES * CPIX    # 33792
RPAD = 130                  # even free dim for DVE 4x mode; col 129 unused
TILES = [4] * 8 + [1]       # chunks per DMA tile (sum = 33)
PRESCALE = 32.0             # host multiplies x by 32 -> squares are 1024*x^2

F32 = mybir.dt.float32
F16 = mybir.dt.float16

_CACHE: dict = {}


def _build_program():
    nc = bacc.Bacc("TRN2", target_bir_lowering=False, debug=False,
                   num_devices=NCORES)

    # x, pre-transposed+scaled+fp16 on host: [chunk, pixel-in-chunk, row]
    x_d = nc.dram_tensor("xt", [NCH, 128, NROW], F16,
                         kind="ExternalInput").ap()
    # idx and wt packed: col c = idx for chunk c, col NCH+c = wt for chunk c
    iw_d = nc.dram_tensor("iw", [128, 2 * NCH], F32,
                          kind="ExternalInput").ap()
    iota_d = nc.dram_tensor("iota", [128, RPAD], F16,
                            kind="ExternalInput").ap()
    out_d = nc.dram_tensor("out", [128, NGRP * MAX_R], F32,
                           kind="ExternalOutput").ap()

    with tile.TileContext(nc) as tc:
        with tc.tile_pool(name="const", bufs=1) as const_pool, \
             tc.tile_pool(name="xin", bufs=3) as xin_pool, \
             tc.tile_pool(name="x2", bufs=3) as x2_pool, \
             tc.tile_pool(name="oh", bufs=8) as oh_pool, \
             tc.tile_pool(name="acc", bufs=1, space="PSUM") as acc_pool:

            accs = [acc_pool.tile([128, RPAD], F32, name=f"acc{g}")
                    for g in range(NGRP)]
            iw_t = const_pool.tile([128, 2 * NCH], F32)
            iota_t = const_pool.tile([128, RPAD], F16)

            c0 = 0
            first = True
            for tch in TILES:
                xin = xin_pool.tile([128, 4, NROW], F16, tag="xin")
                nc.sync.dma_start(
                    xin[:, :tch], x_d[c0:c0 + tch].rearrange("c p n -> p c n"))
                if first:
                    # consts slot in behind the first big load
                    nc.sync.dma_start(iw_t[:], iw_d[:])
                    nc.sync.dma_start(iota_t[:], iota_d[:])
                    first = False
                x2 = x2_pool.tile([128, 4, NROW], F16, tag="x2")
                for j in range(tch):
                    c = c0 + j
                    # per-chunk squares, alternating engines, so chunk c's
                    # matmuls start as soon as its own square lands
                    if c % 2 == 0 and c != NCH - 1:
                        nc.scalar.activation(
                            x2[:, j], xin[:, j],
                            mybir.ActivationFunctionType.Square)
                    else:
                        nc.vector.tensor_tensor(x2[:, j], xin[:, j],
                                                xin[:, j],
                                                op=mybir.AluOpType.mult)
                    oh = oh_pool.tile([128, RPAD], F16, tag="oh")
                    nc.vector.tensor_scalar(
                        oh[:], iota_t[:],
                        scalar1=iw_t[:, c:c + 1],
                        scalar2=iw_t[:, NCH + c:NCH + c + 1],
                        op0=mybir.AluOpType.is_equal,
                        op1=mybir.AluOpType.mult)
                    for g in range(NGRP):
                        nc.tensor.matmul(accs[g][:],
                                         lhsT=x2[:, j, g * 128:(g + 1) * 128],
                                         rhs=oh[:],
                                         start=(c == 0), stop=(c == NCH - 1))
                c0 += tch

            # psum -> sbuf fp16: groups 0-3 on ScalarE, 4-7 on DVE (parallel);
            # each half gets its own output DMA so the second overlaps copies
            res = const_pool.tile([128, NGRP * MAX_R], F32)
            for g in range(4):
                nc.scalar.copy(res[:, g * MAX_R:(g + 1) * MAX_R],
                               accs[g][:, :MAX_R])
            for g in range(4, NGRP):
                nc.vector.tensor_copy(res[:, g * MAX_R:(g + 1) * MAX_R],
                                      accs[g][:, :MAX_R])
            # out DMAs issued from the copy engines' own queues: in-order
            # with the copies, so no cross-engine semaphore hop
            h = 4 * MAX_R
            nc.scalar.dma_start(out_d[:, :h], res[:, :h])
            nc.scalar.dma_start(out_d[:, h:], res[:, h:])

    nc.compile()
    return nc


def _get_program():
    if "nc" not in _CACHE:
        _CACHE["nc"] = _build_program()
    return _CACHE["nc"]


def _host_prep(shell_index: np.ndarray, shells_weight: np.ndarray,
               shells_count: np.ndarray):
    idx_flat = shell_index.reshape(-1).astype(np.int64)
    wt = shells_weight.reshape(-1).astype(np.float64) / (
        shells_count.astype(np.float64)[idx_flat] + EPS)
    idx_pad = np.zeros(NPIX_PAD, np.float32)
    idx_pad[:NPIX] = idx_flat
    wt_pad = np.zeros(NPIX_PAD, np.float32)
    wt_pad[:NPIX] = wt
    # per-core packed [idx | wt], chunk-transposed: A[i, c] = v[c*128 + i]
    iw = []
    for k in range(NCORES):
        sl = slice(k * CPIX, (k + 1) * CPIX)
        iw.append(np.concatenate(
            [idx_pad[sl].reshape(NCH, 128).T,
             wt_pad[sl].reshape(NCH, 128).T], axis=1).astype(np.float32))
    iota = np.broadcast_to(np.arange(RPAD, dtype=np.float16),
                           (128, RPAD)).copy()
    return iw, iota


def kernel(x: np.ndarray, shell_index: np.ndarray,
           shells_weight: np.ndarray, shells_count: np.ndarray,
           _trace: bool = False, **_tr_kwargs) -> np.ndarray:
    assert x.shape == (B, C, S, XDIM)
    nc = _get_program()
    iw, iota = _host_prep(shell_index, shells_weight, shells_count)

    x16 = (x.reshape(NROW, NPIX) * np.float32(PRESCALE)).astype(np.float16)
    in_maps = []
    for k in range(NCORES):
        lo = k * CPIX
        hi = min((k + 1) * CPIX, NPIX)
        xk = np.zeros((CPIX, NROW), np.float16)
        xk[:hi - lo] = x16[:, lo:hi].T
        in_maps.append({"xt": xk.reshape(NCH, 128, NROW), "iw": iw[k],
                        "iota": iota})

    res = run_bass_kernel_spmd(nc, in_maps, list(range(NCORES)),
                               trace=_trace, **_tr_kwargs)
    # each core returns [128, 8*129] f32 partial (1024x scaled) shell sums
    parts = np.stack([res.results[k]["out"] for k in range(NCORES)])
    full = parts.astype(np.float64).sum(axis=0) / (PRESCALE * PRESCALE)
    full = full.reshape(128, NGRP, MAX_R)
    # row-group g holds global rows g*128..(g+1)*128-1
    full = np.ascontiguousarray(full.transpose(1, 0, 2)).reshape(
        NROW, MAX_R).astype(np.float32)
    full = full.reshape(B, C, MAX_R)
    if _trace:
        return full, res
    return full


# revision 12
# speedup vs baseline: 1.0344x; 1.0192x over previous
"""Radial power-spectrum (GroupStat.get_spectrum) Trainium2 kernel.

Math:  out[b,c,r] = sum_{p: idx[p]==r} x[b,c,p]^2 * w[p] / (cnt[r]+eps)

Strategy (8 NeuronCores, sharded over PIXELS, not batch):
  * All B*C = 1024 (b,c) rows on every core; each core owns ~1/8 of the
    33024 pixels (padded to 8*4224 = 33792, pad weight 0).
  * Host prep: transpose x to pixel-major [NPIX, 1024], scale by 32 and
    cast to fp16.  With 1024 rows per pixel the DMA lines are 2 KB, so
    the load runs at full HBM bandwidth AND lands with pixel on the
    partition dim -- no on-device transpose at all.
  * Device pipeline per 128-pixel chunk (33 per core):
      - DMA fp16 tiles [128p, 4, 1024n] (4 chunks per DMA; last tile is
        a single chunk so the post-DMA tail is short)
      - square in fp16 (values are 1024*x^2; the 32x host prescale keeps
        tiny x^2 out of fp16 subnormals), split between ScalarE and DVE
      - DVE: weighted one-hot [128p, 130r] = (iota == idx[p]) * wt[p],
        built ONCE per chunk and reused by all 8 row-groups
      - PE: for each of 8 row-groups g: psum_g[128n,130r] += x2T_g @ oh
  * psum_g -> SBUF f32 (4 copies on ScalarE, 4 on DVE, concurrently),
    two output DMAs overlap the copies; host sums the 8 per-core
    partials (pixel sharding => partial shell sums) and divides by 1024.
"""

import numpy as np

from concourse import bass, bacc, mybir
import concourse.tile as tile
from concourse.bass_utils import run_bass_kernel_spmd

B, C, S, XDIM = 128, 8, 256, 129
MAX_R = XDIM                # 129 shells
EPS = 1e-5
NCORES = 8
NROW = B * C                # 1024 total (b,c) rows
NGRP = NROW // 128          # 8 row-groups of 128
NPIX = S * XDIM             # 33024 pixels
NCH = 33                    # chunks of 128 pixels per core
CPIX = NCH * 128            # 4224 pixels per core
NPIX_PAD = NCORES * CPIX    # 33792
RPAD = 130                  # even free dim for DVE 4x mode; col 129 unused
TILES = [4] * 7 + [2, 2, 1]  # chunks per DMA tile (sum = 33); tapered tail
PRESCALE = 32.0             # host multiplies x by 32 -> squares are 1024*x^2

F32 = mybir.dt.float32
F16 = mybir.dt.float16

_CACHE: dict = {}


def _build_program():
    nc = bacc.Bacc("TRN2", target_bir_lowering=False, debug=False,
                   num_devices=NCORES)

    # x, pre-transposed+scaled+fp16 on host: [chunk, pixel-in-chunk, row]
    x_d = nc.dram_tensor("xt", [NCH, 128, NROW], F16,
                         kind="ExternalInput").ap()
    # idx and wt packed: col c = idx for chunk c, col NCH+c = wt for chunk c
    iw_d = nc.dram_tensor("iw", [128, 2 * NCH], F32,
                          kind="ExternalInput").ap()
    iota_d = nc.dram_tensor("iota", [128, RPAD], F16,
                            kind="ExternalInput").ap()
    out_d = nc.dram_tensor("out", [128, NGRP * MAX_R], F32,
                           kind="ExternalOutput").ap()

    with tile.TileContext(nc) as tc:
        with tc.tile_pool(name="const", bufs=1) as const_pool, \
             tc.tile_pool(name="xin", bufs=3) as xin_pool, \
             tc.tile_pool(name="x2", bufs=3) as x2_pool, \
             tc.tile_pool(name="oh", bufs=8) as oh_pool, \
             tc.tile_pool(name="acc", bufs=1, space="PSUM") as acc_pool:

            # one PSUM tile, one 2KB bank per row-group (512 f32; cols
            # 130.. unused) so the final copies can batch across groups
            acc = acc_pool.tile([128, NGRP, 512], F32)
            iw_t = const_pool.tile([128, 2 * NCH], F32)
            iota_t = const_pool.tile([128, RPAD], F16)

            c0 = 0
            first = True
            for tch in TILES:
                xin = xin_pool.tile([128, 4, NROW], F16, tag="xin")
                nc.sync.dma_start(
                    xin[:, :tch], x_d[c0:c0 + tch].rearrange("c p n -> p c n"))
                if first:
                    # consts slot in behind the first big load
                    nc.sync.dma_start(iw_t[:], iw_d[:])
                    nc.sync.dma_start(iota_t[:], iota_d[:])
                    first = False
                x2 = x2_pool.tile([128, 4, NROW], F16, tag="x2")
                for j in range(tch):
                    c = c0 + j
                    # per-chunk squares, alternating engines, so chunk c's
                    # matmuls start as soon as its own square lands
                    if c % 2 == 0 and c != NCH - 1:
                        nc.scalar.activation(
                            x2[:, j], xin[:, j],
                            mybir.ActivationFunctionType.Square)
                    else:
                        nc.vector.tensor_tensor(x2[:, j], xin[:, j],
                                                xin[:, j],
                                                op=mybir.AluOpType.mult)
                    oh = oh_pool.tile([128, RPAD], F16, tag="oh")
                    nc.vector.tensor_scalar(
                        oh[:], iota_t[:],
                        scalar1=iw_t[:, c:c + 1],
                        scalar2=iw_t[:, NCH + c:NCH + c + 1],
                        op0=mybir.AluOpType.is_equal,
                        op1=mybir.AluOpType.mult)
                    for g in range(NGRP):
                        nc.tensor.matmul(acc[:, g, :RPAD],
                                         lhsT=x2[:, j, g * 128:(g + 1) * 128],
                                         rhs=oh[:],
                                         start=(c == 0), stop=(c == NCH - 1))
                c0 += tch

            # psum -> sbuf: groups 0-3 in one strided ScalarE copy, 4-7 in
            # one DVE copy (parallel); each half gets its own output DMA
            res = const_pool.tile([128, NGRP * MAX_R], F32)
            h = 4 * MAX_R
            res3a = res[:, :h].rearrange("p (g r) -> p g r", g=4)
            res3b = res[:, h:].rearrange("p (g r) -> p g r", g=4)
            nc.scalar.copy(res3a, acc[:, 0:4, :MAX_R])
            nc.vector.tensor_copy(res3b, acc[:, 4:NGRP, :MAX_R])
            nc.sync.dma_start(out_d[:, :h], res[:, :h])
            nc.sync.dma_start(out_d[:, h:], res[:, h:])

    nc.compile()
    return nc


def _get_program():
    if "nc" not in _CACHE:
        _CACHE["nc"] = _build_program()
    return _CACHE["nc"]


def _host_prep(shell_index: np.ndarray, shells_weight: np.ndarray,
               shells_count: np.ndarray):
    idx_flat = shell_index.reshape(-1).astype(np.int64)
    wt = shells_weight.reshape(-1).astype(np.float64) / (
        shells_count.astype(np.float64)[idx_flat] + EPS)
    idx_pad = np.zeros(NPIX_PAD, np.float32)
    idx_pad[:NPIX] = idx_flat
    wt_pad = np.zeros(NPIX_PAD, np.float32)
    wt_pad[:NPIX] = wt
    # per-core packed [idx | wt], chunk-transposed: A[i, c] = v[c*128 + i]
    iw = []
    for k in range(NCORES):
        sl = slice(k * CPIX, (k + 1) * CPIX)
        iw.append(np.concatenate(
            [idx_pad[sl].reshape(NCH, 128).T,
             wt_pad[sl].reshape(NCH, 128).T], axis=1).astype(np.float32))
    iota = np.broadcast_to(np.arange(RPAD, dtype=np.float16),
                           (128, RPAD)).copy()
    return iw, iota


def kernel(x: np.ndarray, shell_index: np.ndarray,
           shells_weight: np.ndarray, shells_count: np.ndarray,
           _trace: bool = False, **_tr_kwargs) -> np.ndarray:
    assert x.shape == (B, C, S, XDIM)
    nc = _get_program()
    iw, iota = _host_prep(shell_index, shells_weight, shells_count)

    x16 = (x.reshape(NROW, NPIX) * np.float32(PRESCALE)).astype(np.float16)
    in_maps = []
    for k in range(NCORES):
        lo = k * CPIX
        hi = min((k + 1) * CPIX, NPIX)
        xk = np.zeros((CPIX, NROW), np.float16)
        xk[:hi - lo] = x16[:, lo:hi].T
        in_maps.append({"xt": xk.reshape(NCH, 128, NROW), "iw": iw[k],
                        "iota": iota})

    res = run_bass_kernel_spmd(nc, in_maps, list(range(NCORES)),
                               trace=_trace, **_tr_kwargs)
    # each core returns [128, 8*129] f32 partial (1024x scaled) shell sums
    parts = np.stack([res.results[k]["out"] for k in range(NCORES)])
    full = parts.astype(np.float64).sum(axis=0) / (PRESCALE * PRESCALE)
    full = full.reshape(128, NGRP, MAX_R)
    # row-group g holds global rows g*128..(g+1)*128-1
    full = np.ascontiguousarray(full.transpose(1, 0, 2)).reshape(
        NROW, MAX_R).astype(np.float32)
    full = full.reshape(B, C, MAX_R)
    if _trace:
        return full, res
    return full
